# revision 14
# baseline (speedup 1.0000x reference)
"""EdgeConv encoder for Trainium2 (Bass/Tile), v2.

Math (one EdgeConv layer, PyG semantics, aggr='add' over dst):
  u[n]  = x[n] @ (A_i - A_j).T + ba     (node-level)   A_i|A_j = wa split
  v[n]  = x[n] @ A_j.T                  (node-level)
  t_e   = relu(u[dst_e] + v[src_e])     (edge-level)
  agg[n] = sum_{e: dst_e = n} t_e       (scatter-add)
  conv[n] = agg[n] @ wb2 + deg[n] * c0  (node-level; BN+linear folded)
  layer1: h = l2norm(relu(conv)); layer2: out = conv

Sharding: edges partitioned by dst across 8 cores (each core owns 49
contiguous 128-node blocks); outputs disjoint; v computed redundantly.

v2 design vs v0:
  - v kept resident in SBUF node-major ([128, 392*128] bf16); v[src]
    fetched with SBUF-source transposed dma_gather -> vgT[c, e] in
    feature-major layout (no HBM round-trip, no 256B-random HBM reads).
  - message built feature-major with WIDE matmuls: one [C, 4C] matmul
    per 4-chunk run (lhsT = u_blk) plus one identity matmul adding vgT,
    one ACT relu per run.
  - per-chunk PE transpose (is_transpose matmul, bf16 PSUM) flips
    t^T[c,e] -> t[e,c] for the scatter matmul agg^T += t^T S.
  - conv computed node-major (lhsT=aggT, rhs=wb2) so the L1 row norm is
    per-partition: ACT relu -> DVE tensor_tensor_reduce (sum h^2) ->
    sqrt/max/recip -> ACT scale. Output written node-major [N, C].
  - node phase writes v/u straight to SBUF (no DRAM scratch, no
    392 small DMA writes).
"""

import sys

sys.path.insert(0, "/opt/trn_rl_repo")

import numpy as np

from concourse import bacc, bass, mybir, tile

F32 = mybir.dt.float32
BF16 = mybir.dt.bfloat16
I16 = mybir.dt.int16
BF16_NP = mybir.dt.np(BF16)
AF = mybir.ActivationFunctionType

C = 128
GRP = 4            # chunks per message run (psum width 512)
MAXCH = 24         # chunks per dma_gather call; >6 calls in flight races
SPKT = False       # single_packet corrupts gathers on this runtime
SPLIT = 32768      # int16 index split point (nodes >= SPLIT use offset base)


def build_layer(n_blocks_total: int, blocks_per_core: int,
                sched_lo: list[int], sched_hi: list[int],
                apply_norm: bool, node_grp: int = 8, gather_blocks: int = 3):
    NBT, BPC = n_blocks_total, blocks_per_core
    import os as _os
    ablate = int(_os.environ.get("EDGECONV_ABLATE", "0"))
    blocks, groups, TC = make_layout(sched_lo, sched_hi, BPC, gather_blocks)
    nc = bacc.Bacc("TRN2", num_swdge_queues=4)

    # ---- inputs ----
    xt = nc.declare_dram_parameter("xt", [C, NBT * C], BF16, isOutput=False)
    xt_own = nc.declare_dram_parameter("xt_own", [C, BPC * C], BF16, isOutput=False)
    wv_t = nc.declare_dram_parameter("wv_t", [C, C], BF16, isOutput=False)
    wu_t = nc.declare_dram_parameter("wu_t", [C, C], BF16, isOutput=False)
    ba = nc.declare_dram_parameter("ba", [1, C], BF16, isOutput=False)
    wb2 = nc.declare_dram_parameter("wb2", [C, C], BF16, isOutput=False)
    c0 = nc.declare_dram_parameter("c0", [1, C], BF16, isOutput=False)
    iota_col = nc.declare_dram_parameter("iota_col", [C, 1], F32, isOutput=False)
    iota_row4 = nc.declare_dram_parameter("iota_row4", [C, GRP, C], BF16, isOutput=False)
    ident = nc.declare_dram_parameter("ident", [C, C], BF16, isOutput=False)
    deg = nc.declare_dram_parameter("deg", [1, BPC * C], BF16, isOutput=False)
    ones_row = nc.declare_dram_parameter("ones_row", [1, C], BF16, isOutput=False)
    src16 = nc.declare_dram_parameter("src16", [128, TC * 8], I16, isOutput=False)
    dst_row = nc.declare_dram_parameter("dst_row", [1, TC * C], BF16, isOutput=False)
    dst_col = nc.declare_dram_parameter("dst_col", [C, TC], BF16, isOutput=False)
    out_nm = nc.declare_dram_parameter("out_nm", [BPC * C, C], F32, isOutput=True)

    max_gchunks = max((nlo + nhi for _, nlo, nhi, _ in groups), default=1)

    with tile.TileContext(nc) as tc:
        with (
            tc.tile_pool(name="persist", bufs=1) as persist,
            tc.tile_pool(name="nodeio", bufs=2) as nodeio,
            tc.tile_pool(name="edgeio", bufs=2) as edgeio,
            tc.tile_pool(name="sbuild", bufs=3) as sbuild,
            tc.tile_pool(name="tbuf", bufs=3) as tbuf,
            tc.tile_pool(name="outio", bufs=2) as outio,
            tc.tile_pool(name="widep", bufs=3, space="PSUM") as widep,
            tc.tile_pool(name="tpp", bufs=2, space="PSUM") as tpp,
            tc.tile_pool(name="aggp", bufs=1, space="PSUM") as aggp,
        ):
            # ---- persistent SBUF state ----
            nlo_elems = min(SPLIT, NBT * C)
            nhi_elems = max(NBT * C - nlo_elems, 0)
            v_lo = persist.tile([128, nlo_elems], BF16, tag="vlo")
            v_hi = persist.tile([128, max(nhi_elems, C)], BF16, tag="vhi",
                                name="v_hi")

            def v_dst(q0, q1):
                """SBUF destination APs covering node blocks [q0, q1)."""
                e0, e1 = q0 * C, q1 * C
                parts = []
                if e0 < nlo_elems:
                    parts.append(v_lo[:, e0: min(e1, nlo_elems)])
                if e1 > nlo_elems:
                    parts.append(v_hi[:, max(e0 - nlo_elems, 0): e1 - nlo_elems])
                return parts

            u_sb = persist.tile([128, BPC * C], BF16, tag="u")
            srci_sb = persist.tile([128, TC * 8], I16, tag="srci")
            nc.sync.dma_start(out=srci_sb[:], in_=src16[:])
            dstc_sb = persist.tile([C, TC], BF16, tag="dstc")
            nc.sync.dma_start(out=dstc_sb[:], in_=dst_col[:])
            wv_sb = persist.tile([C, C], BF16, tag="wv")
            nc.sync.dma_start(out=wv_sb[:], in_=wv_t[:])
            wu_sb = persist.tile([C, C], BF16, tag="wu")
            nc.sync.dma_start(out=wu_sb[:], in_=wu_t[:])
            ba_sb = persist.tile([1, C], BF16, tag="ba")
            nc.sync.dma_start(out=ba_sb[:], in_=ba[:])
            wb2_sb = persist.tile([C, C], BF16, tag="wb2")
            nc.sync.dma_start(out=wb2_sb[:], in_=wb2[:])
            c0_sb = persist.tile([1, C], BF16, tag="c0")
            nc.sync.dma_start(out=c0_sb[:], in_=c0[:])
            ic_sb = persist.tile([C, 1], F32, tag="ic")
            nc.sync.dma_start(out=ic_sb[:], in_=iota_col[:])
            ir_sb = persist.tile([C, GRP, C], BF16, tag="ir")
            nc.sync.dma_start(out=ir_sb[:], in_=iota_row4[:])
            id_sb = persist.tile([C, C], BF16, tag="id")
            nc.sync.dma_start(out=id_sb[:], in_=ident[:])
            deg_sb = persist.tile([1, BPC * C], BF16, tag="deg")
            nc.sync.dma_start(out=deg_sb[:], in_=deg[:])
            onesr_sb = persist.tile([1, C], BF16, tag="onesr")
            nc.sync.dma_start(out=onesr_sb[:], in_=ones_row[:])

            # ================= node phase =================
            # v[n] for ALL nodes -> v_sb (node-major: partition n%128,
            # free bytes (n//128)*256). u[n] for own nodes -> u_sb.
            for g0 in range(0, NBT, node_grp):
                g1 = min(g0 + node_grp, NBT)
                xt_sb = nodeio.tile([C, node_grp * C], BF16, tag="xt")
                nc.sync.dma_start(out=xt_sb[:, : (g1 - g0) * C],
                                  in_=xt[:, g0 * C: g1 * C])
                for q0 in range(g0, g1, GRP):
                    q1 = min(q0 + GRP, g1)
                    vps = widep.tile([C, GRP * C], F32, tag="wide")
                    for b in range(q0, q1):
                        lhs = xt_sb[:, (b - g0) * C: (b - g0 + 1) * C]
                        nc.tensor.matmul(vps[:, (b - q0) * C: (b - q0 + 1) * C],
                                         lhsT=lhs, rhs=wv_sb[:],
                                         start=True, stop=True)
                    off = 0
                    for part in v_dst(q0, q1):
                        w = part.shape[-1]
                        if (q0 // GRP) % 2 == 0:
                            nc.vector.tensor_copy(
                                out=part, in_=vps[:, off: off + w])
                        else:
                            nc.scalar.activation(
                                out=part, in_=vps[:, off: off + w],
                                func=AF.Copy)
                        off += w

            for g0 in range(0, BPC, node_grp):
                g1 = min(g0 + node_grp, BPC)
                xo_sb = nodeio.tile([C, node_grp * C], BF16, tag="xo")
                nc.sync.dma_start(out=xo_sb[:, : (g1 - g0) * C],
                                  in_=xt_own[:, g0 * C: g1 * C])
                for q0 in range(g0, g1, GRP):
                    q1 = min(q0 + GRP, g1)
                    ups = widep.tile([C, GRP * C], F32, tag="wide")
                    for b in range(q0, q1):
                        lhs = xo_sb[:, (b - g0) * C: (b - g0 + 1) * C]
                        sl = slice((b - q0) * C, (b - q0 + 1) * C)
                        nc.tensor.matmul(ups[:, sl], lhsT=lhs, rhs=wu_sb[:],
                                         start=True, stop=False)
                        nc.tensor.matmul(ups[:, sl], lhsT=onesr_sb[:],
                                         rhs=ba_sb[:], start=False, stop=True)
                    nc.vector.tensor_copy(out=u_sb[:, q0 * C: q1 * C],
                                          in_=ups[:, : (q1 - q0) * C])

            # ================= edge phase =================
            grp_of_block = {}
            for gi_, (g_start, nlo_g, nhi_g, bs) in enumerate(groups):
                grp_of_block[bs[0]] = gi_

            vg_sb = None
            vg_base = 0
            gq = [0]

            # software pipeline: defer transpose+scatter of the previous
            # run while the next run's message matmul issues.
            pending = []

            def flush_pending():
                while pending:
                    pending.pop(0)()

            for b in range(BPC):
                lo0, nl, hi0, nh = blocks[b]
                nch = nl + nh
                if b in grp_of_block:
                    g_start, nlo_g, nhi_g, _ = groups[grp_of_block[b]]
                    vg_base = g_start
                    ng = nlo_g + nhi_g
                    if ng > 0:
                        vg_sb = edgeio.tile([128, max_gchunks * C], BF16,
                                            tag="vg")
                        if ablate == 1:
                            nc.gpsimd.memset(vg_sb[:], 0.0)
                        else:
                         for (cb, cn, lo_half) in ((0, nlo_g, True),
                                                  (nlo_g, nhi_g, False)):
                            in_ap = v_lo[:] if lo_half else v_hi[:]
                            for c0_ in range(0, cn, MAXCH):
                                cw = min(MAXCH, cn - c0_)
                                sl = g_start + cb + c0_
                                o0 = (cb + c0_) * C
                                nc.gpsimd.dma_gather(
                                    out_ap=vg_sb[:, o0: o0 + cw * C]
                                        .unsqueeze(1),
                                    in_ap=in_ap,
                                    idxs_ap=srci_sb[:, sl * 8: (sl + cw) * 8],
                                    num_idxs=cw * C,
                                    num_idxs_reg=cw * C,
                                    elem_size=C,
                                    transpose=True,
                                    sbuf_tokens_per_rank=128,
                                    sbuf_free_dim_per_rank=256,
                                    single_packet=SPKT,
                                    queue_num=gq[0] % 4)
                                gq[0] += 1
                        dstrg_sb = edgeio.tile([1, max_gchunks * C], BF16,
                                               tag="dstr")
                        nc.sync.dma_start(
                            out=dstrg_sb[:, : ng * C],
                            in_=dst_row[0:1, g_start * C: (g_start + ng) * C])

                if nch == 0:
                    flush_pending()
                    agg_sb = outio.tile([C, C], BF16, tag="aggsb")
                    nc.gpsimd.memset(agg_sb[:], 0.0)
                else:
                    aggT = aggp.tile([C, C], F32, tag="agg")
                    ch_done = [0]
                    for (r0, rn) in ((lo0, nl), (hi0, nh)):
                        for j0 in range(0, rn, GRP):
                            gw = min(GRP, rn - j0)
                            rel = r0 + j0 - vg_base
                            slot0 = r0 + j0
                            # one-hot builds for this run
                            bc_ps = widep.tile([C, GRP * C], F32, tag="wide")
                            nc.tensor.matmul(
                                bc_ps[:, : gw * C], lhsT=onesr_sb[:],
                                rhs=dstrg_sb[0:1, rel * C: (rel + gw) * C],
                                start=True, stop=True)
                            sT_sb = sbuild.tile([C, GRP * C], BF16, tag="sT")
                            nc.vector.tensor_scalar(
                                out=sT_sb[:, : gw * C],
                                in0=bc_ps[:, : gw * C],
                                scalar1=ic_sb[:], scalar2=None,
                                op0=mybir.AluOpType.is_equal)
                            s_sb = sbuild.tile([C, GRP, C], BF16, tag="s")
                            nc.vector.tensor_tensor(
                                out=s_sb[:, :gw, :],
                                in0=ir_sb[:, :gw, :],
                                in1=dstc_sb[:, slot0: slot0 + gw]
                                    .to_broadcast([C, gw, C]),
                                op=mybir.AluOpType.is_equal)
                            # message (feature-major): u[dst] + v[src]
                            msg_ps = widep.tile([C, GRP * C], F32, tag="wide")
                            nc.tensor.matmul(
                                msg_ps[:, : gw * C],
                                lhsT=u_sb[:, b * C: (b + 1) * C],
                                rhs=sT_sb[:, : gw * C],
                                start=True, stop=False)
                            nc.tensor.matmul(
                                msg_ps[:, : gw * C], lhsT=id_sb[:],
                                rhs=vg_sb[:, rel * C: (rel + gw) * C],
                                start=False, stop=True)
                            tT_sb = tbuf.tile([C, GRP * C], BF16, tag="tT")
                            nc.scalar.activation(out=tT_sb[:, : gw * C],
                                                 in_=msg_ps[:, : gw * C],
                                                 func=AF.Relu)

                            def finish(tT_sb=tT_sb, s_sb=s_sb, gw=gw,
                                       aggT=aggT, ch_done=ch_done, nch=nch):
                                t_sb = tbuf.tile([C, GRP * C], BF16, tag="t")
                                if ablate == 2:
                                    nc.vector.tensor_copy(
                                        out=t_sb[:, : gw * C],
                                        in_=tT_sb[:, : gw * C])
                                else:
                                    tp_ps = tpp.tile([C, GRP * C], BF16, tag="tp")
                                    for j in range(gw):
                                        nc.tensor.matmul(
                                            tp_ps[:, j * C: (j + 1) * C],
                                            lhsT=tT_sb[:, j * C: (j + 1) * C],
                                            rhs=id_sb[:], is_transpose=True,
                                            start=True, stop=True)
                                    nc.vector.tensor_copy(out=t_sb[:, : gw * C],
                                                          in_=tp_ps[:, : gw * C])
                                for j in range(gw):
                                    nc.tensor.matmul(
                                        aggT[:],
                                        lhsT=t_sb[:, j * C: (j + 1) * C],
                                        rhs=s_sb[:, j, :],
                                        start=(ch_done[0] == 0),
                                        stop=(ch_done[0] == nch - 1))
                                    ch_done[0] += 1

                            flush_pending()
                            pending.append(finish)
                    flush_pending()
                    agg_sb = outio.tile([C, C], BF16, tag="aggsb")
                    nc.vector.tensor_copy(out=agg_sb[:], in_=aggT[:])

                # conv: node-major [n, c]
                cps = aggp.tile([C, C], F32, tag="conv")
                nc.tensor.matmul(cps[:], lhsT=agg_sb[:], rhs=wb2_sb[:],
                                 start=True, stop=False)
                nc.tensor.matmul(cps[:], lhsT=deg_sb[0:1, b * C: (b + 1) * C],
                                 rhs=c0_sb[:], start=False, stop=True)

                o_sb = outio.tile([C, C], F32, tag="o")
                if apply_norm:
                    h_sb = outio.tile([C, C], F32, tag="h")
                    nc.scalar.activation(out=h_sb[:], in_=cps[:],
                                         func=AF.Relu)
                    sq_sb = outio.tile([C, C], BF16, tag="sq")
                    nrm = outio.tile([C, 4], F32, tag="nrm")
                    nc.scalar.activation(out=sq_sb[:], in_=h_sb[:],
                                         func=AF.Square,
                                         accum_out=nrm[:, 0:1])
                    nc.scalar.activation(out=nrm[:, 1:2], in_=nrm[:, 0:1],
                                         func=AF.Sqrt)
                    nc.vector.tensor_scalar(out=nrm[:, 2:3], in0=nrm[:, 1:2],
                                            scalar1=1e-12, scalar2=None,
                                            op0=mybir.AluOpType.max)
                    nc.vector.reciprocal(out=nrm[:, 3:4], in_=nrm[:, 2:3])
                    nc.scalar.activation(out=o_sb[:], in_=h_sb[:],
                                         func=AF.Copy, scale=nrm[:, 3:4])
                else:
                    nc.scalar.activation(out=o_sb[:], in_=cps[:],
                                         func=AF.Copy)
                nc.sync.dma_start(out=out_nm[b * C: (b + 1) * C, :],
                                  in_=o_sb[:])

    nc.compile()
    return nc


# ---------------- host-side data prep ----------------


def make_layout(sched_lo, sched_hi, bpc, gather_blocks=3):
    """Group-major slot order: per gather group, all lo slots (block-major)
    then all hi slots. Returns per-block (lo_start, nlo, hi_start, nhi),
    group list (chunk_start, nlo_g, nhi_g, blocks)."""
    blocks = []
    groups = []
    pos = 0
    b = 0
    while b < bpc:
        bs = list(range(b, min(b + gather_blocks, bpc)))
        g_start = pos
        lo_starts = {}
        for bb in bs:
            lo_starts[bb] = pos
            pos += sched_lo[bb]
        nlo_g = pos - g_start
        hi_starts = {}
        for bb in bs:
            hi_starts[bb] = pos
            pos += sched_hi[bb]
        nhi_g = pos - g_start - nlo_g
        for bb in bs:
            blocks.append((lo_starts[bb], sched_lo[bb],
                           hi_starts[bb], sched_hi[bb]))
        groups.append((g_start, nlo_g, nhi_g, bs))
        b += gather_blocks
    return blocks, groups, pos


def prep_edges(src, dst, n_cores, bpc, gather_blocks=3):
    """Partition edges by dst core/block, split each block's edges into
    lo (src < SPLIT) and hi chunks for int16 dma_gather indexing."""
    npc = bpc * C
    order = np.argsort(dst, kind="stable")
    src_s, dst_s = src[order], dst[order]
    core_lists = []
    nlo = np.zeros((n_cores, bpc), np.int64)
    nhi = np.zeros((n_cores, bpc), np.int64)
    for k in range(n_cores):
        lo_ = np.searchsorted(dst_s, k * npc, side="left")
        hi_ = np.searchsorted(dst_s, (k + 1) * npc, side="left")
        s_k, d_k = src_s[lo_:hi_], dst_s[lo_:hi_] - k * npc
        blk = d_k // C
        per_blk = []
        for b in range(bpc):
            m = blk == b
            sb, db = s_k[m], d_k[m] - b * C
            isl = sb < SPLIT
            per_blk.append(((sb[isl], db[isl]), (sb[~isl], db[~isl])))
            nlo[k, b] = isl.sum()
            nhi[k, b] = (~isl).sum()
        core_lists.append(per_blk)
    sched_lo = [int(x) for x in np.ceil(nlo.max(axis=0) / C).astype(np.int64)]
    sched_hi = [int(x) for x in np.ceil(nhi.max(axis=0) / C).astype(np.int64)]
    blocks, groups, TC = make_layout(sched_lo, sched_hi, bpc, gather_blocks)

    per_core = []
    for k in range(n_cores):
        si16 = np.zeros((16, TC * 8), np.int16)
        db_ = np.full((TC, C), 200.0, np.float64)
        for b in range(bpc):
            (slo, sdlo), (shi, sdhi) = core_lists[k][b]
            lo0, nl, hi0, nh = blocks[b]
            for (vals, dvals, base, nslots, off) in (
                    (slo, sdlo, lo0, nl, 0), (shi, sdhi, hi0, nh, SPLIT)):
                n = len(vals)
                if nslots == 0:
                    continue
                idx = np.arange(n)
                ch = base + idx // C
                lane = idx % C
                iv = (vals - off).astype(np.int16)
                si16[lane % 16, ch * 8 + lane // 16] = iv
                db_[ch, lane] = dvals
        full = np.zeros((128, TC * 8), np.int16)
        for rr in range(8):
            full[rr * 16: (rr + 1) * 16] = si16
        per_core.append({
            "src16": full,
            "dst_col": np.ascontiguousarray(db_.T.astype(BF16_NP)),
            "dst_row": np.ascontiguousarray(
                db_.reshape(1, -1).astype(BF16_NP)),
        })
    return sched_lo, sched_hi, per_core


def fold_weights(wa, ba_, g, be, rm, rv, wb, bb, bn_eps=1e-5):
    wa = wa.astype(np.float64)
    A_i, A_j = wa[:, :C], wa[:, C:]
    s = g.astype(np.float64) / np.sqrt(rv.astype(np.float64) + bn_eps)
    wb64 = wb.astype(np.float64)
    wu_t = (A_i - A_j).T
    wv_t = A_j.T
    wb2 = s[:, None] * wb64.T          # wb2[j, i] = s_j * wb[i, j]
    c0 = bb.astype(np.float64) + (be.astype(np.float64) - rm.astype(np.float64) * s) @ wb64.T
    return (wu_t.astype(BF16_NP), wv_t.astype(BF16_NP),
            ba_.astype(BF16_NP).reshape(1, C),
            wb2.astype(BF16_NP), c0.astype(BF16_NP).reshape(1, C))


def make_consts():
    ic = np.arange(C, dtype=np.float32).reshape(C, 1)
    ir4 = np.tile(np.arange(C, dtype=np.float64), (C, GRP, 1)).astype(BF16_NP)
    ident = np.eye(C, dtype=np.float64).astype(BF16_NP)
    return ic, ir4, ident


# ======================================================================
# Full-problem kernel: 2-layer EdgeConv encoder, N=50000, E=600000, C=128
# ======================================================================

import os

N_NODES = 50000
N_EDGES = 600000
CORES = 8
BPC = 49                  # blocks per core
NBT = CORES * BPC         # 392 blocks total
NP = NBT * C              # padded node count 50176
BN_EPS = 1e-5

LAST = {}                 # timing/info stash for test harness


def _prep_all(x, edge_index):
    src = np.asarray(edge_index[0], np.int64).astype(np.int32)
    dst = np.asarray(edge_index[1], np.int64).astype(np.int32)
    sched_lo, sched_hi, per_core = prep_edges(src, dst, CORES, BPC)
    deg_full = np.bincount(dst, minlength=NP).astype(np.float64)
    x_pad = np.zeros((NP, C), np.float32)
    x_pad[:N_NODES] = x
    xt = np.ascontiguousarray(x_pad.T).astype(BF16_NP)
    return sched_lo, sched_hi, per_core, deg_full, xt


def _layer_inputs(xt_bf16, per_core, deg_full, wset):
    wu_t, wv_t, ba_f, wb2, c0 = wset
    ic, ir4, ident = make_consts()
    onesr = np.ones((1, C), dtype=BF16_NP)
    in_maps = []
    for k in range(CORES):
        npc = BPC * C
        in_maps.append({
            "xt": xt_bf16,
            "xt_own": np.ascontiguousarray(xt_bf16[:, k * npc: (k + 1) * npc]),
            "wv_t": wv_t, "wu_t": wu_t, "ba": ba_f, "wb2": wb2, "c0": c0,
            "iota_col": ic, "iota_row4": ir4, "ident": ident,
            "deg": np.ascontiguousarray(
                deg_full[k * npc: (k + 1) * npc].reshape(1, npc).astype(BF16_NP)),
            "ones_row": onesr,
            "src16": per_core[k]["src16"],
            "dst_row": per_core[k]["dst_row"],
            "dst_col": per_core[k]["dst_col"],
        })
    return in_maps


_NTFF_HOOK = None


def _get_ntff_hook():
    global _NTFF_HOOK
    if _NTFF_HOOK is None:
        sys.path.insert(0, "/root/.axon_site")
        from trn_agent_boot.trn_boot import _ntff_profile_via_ctypes
        _NTFF_HOOK = _ntff_profile_via_ctypes("/opt/axon/libaxon_pjrt.so")
    return _NTFF_HOOK


def _run(nc, in_maps):
    import tempfile
    from concourse import bass2jax
    trace = bool(int(os.environ.get("EDGECONV_TRACE", "0")))
    hook = _get_ntff_hook() if trace else None
    if hook is None:
        results = bass2jax.run_bass_via_pjrt(nc, in_maps, n_cores=CORES)
        LAST.setdefault("exec_ns", []).append(None)
        return results
    neff_dir = tempfile.mkdtemp(prefix="edgeconv_ntff_")
    with hook(neff_dir, [0]):
        results = bass2jax.run_bass_via_pjrt(nc, in_maps, n_cores=CORES)
    exec_ns = None
    try:
        import glob as _glob
        import gauge.profiler
        from concourse._compat import FishPath
        if _glob.glob(os.path.join(neff_dir, "*_body*.ntff")):
            profile = gauge.profiler.Profile(
                profile_path=FishPath(neff_dir), kernel_dev_mode=True,
                profile_on_exit=False, bass_kernel=nc.m,
                offline_processing=True, fname="*_body*")
            pr = profile.to_perfetto(model_index=(0,))
            if pr:
                exec_ns = pr[0].exec_time_ns
                LAST.setdefault("trace_paths", []).append(pr[0].trace_path)
    except Exception as e:  # profiling must never break the kernel
        LAST.setdefault("trace_errors", []).append(repr(e))
    LAST.setdefault("neff_dirs", []).append(neff_dir)
    LAST.setdefault("exec_ns", []).append(exec_ns)
    return results


def kernel(**inputs):
    x = np.asarray(inputs["x"], np.float32)
    edge_index = np.asarray(inputs["edge_index"])
    sched_lo, sched_hi, per_core, deg_full, xt = _prep_all(x, edge_index)

    w1 = fold_weights(np.asarray(inputs["w1a"]), np.asarray(inputs["b1a"]),
                      np.asarray(inputs["g1"]), np.asarray(inputs["be1"]),
                      np.asarray(inputs["rm1"]), np.asarray(inputs["rv1"]),
                      np.asarray(inputs["w1b"]), np.asarray(inputs["b1b"]),
                      BN_EPS)
    w2 = fold_weights(np.asarray(inputs["w2a"]), np.asarray(inputs["b2a"]),
                      np.asarray(inputs["g2"]), np.asarray(inputs["be2"]),
                      np.asarray(inputs["rm2"]), np.asarray(inputs["rv2"]),
                      np.asarray(inputs["w2b"]), np.asarray(inputs["b2b"]),
                      BN_EPS)

    nc1 = build_layer(NBT, BPC, sched_lo, sched_hi, apply_norm=True)
    r1 = _run(nc1, _layer_inputs(xt, per_core, deg_full, w1))
    h = np.concatenate([np.asarray(r["out_nm"], np.float32) for r in r1],
                       axis=0)                        # [NP, C] node-major
    xt2 = np.ascontiguousarray(h.T).astype(BF16_NP)   # [C, NP] feature-major

    nc2 = build_layer(NBT, BPC, sched_lo, sched_hi, apply_norm=False)
    r2 = _run(nc2, _layer_inputs(xt2, per_core, deg_full, w2))
    out = np.concatenate([np.asarray(r["out_nm"], np.float32) for r in r2],
                         axis=0)
    return np.ascontiguousarray(out[:N_NODES]).astype(np.float32)


# revision 15
# speedup vs baseline: 1.2314x; 1.2314x over previous
"""EdgeConv encoder for Trainium2 (Bass/Tile), v2.

Math (one EdgeConv layer, PyG semantics, aggr='add' over dst):
  u[n]  = x[n] @ (A_i - A_j).T + ba     (node-level)   A_i|A_j = wa split
  v[n]  = x[n] @ A_j.T                  (node-level)
  t_e   = relu(u[dst_e] + v[src_e])     (edge-level)
  agg[n] = sum_{e: dst_e = n} t_e       (scatter-add)
  conv[n] = agg[n] @ wb2 + deg[n] * c0  (node-level; BN+linear folded)
  layer1: h = l2norm(relu(conv)); layer2: out = conv

Sharding: edges partitioned by dst across 8 cores (each core owns 49
contiguous 128-node blocks); outputs disjoint; v computed redundantly.

v2 design vs v0:
  - v kept resident in SBUF node-major ([128, 392*128] bf16); v[src]
    fetched with SBUF-source transposed dma_gather -> vgT[c, e] in
    feature-major layout (no HBM round-trip, no 256B-random HBM reads).
  - message built feature-major with WIDE matmuls: one [C, 4C] matmul
    per 4-chunk run (lhsT = u_blk) plus one identity matmul adding vgT,
    one ACT relu per run.
  - per-chunk PE transpose (is_transpose matmul, bf16 PSUM) flips
    t^T[c,e] -> t[e,c] for the scatter matmul agg^T += t^T S.
  - conv computed node-major (lhsT=aggT, rhs=wb2) so the L1 row norm is
    per-partition: ACT relu -> DVE tensor_tensor_reduce (sum h^2) ->
    sqrt/max/recip -> ACT scale. Output written node-major [N, C].
  - node phase writes v/u straight to SBUF (no DRAM scratch, no
    392 small DMA writes).
"""

import sys

sys.path.insert(0, "/opt/trn_rl_repo")

import numpy as np

from concourse import bacc, bass, mybir, tile

F32 = mybir.dt.float32
BF16 = mybir.dt.bfloat16
I16 = mybir.dt.int16
BF16_NP = mybir.dt.np(BF16)
AF = mybir.ActivationFunctionType

C = 128
GRP = 4            # chunks per message run (psum width 512)
MAXCH = 24         # chunks per dma_gather call; >6 calls in flight races
SPKT = False       # single_packet corrupts gathers on this runtime
SPLIT = 32768      # int16 index split point (nodes >= SPLIT use offset base)


def build_layer(n_blocks_total: int, blocks_per_core: int,
                sched_lo: list[int], sched_hi: list[int],
                apply_norm: bool, node_grp: int = 16, gather_blocks: int = 2):
    NBT, BPC = n_blocks_total, blocks_per_core
    import os as _os
    ablate = int(_os.environ.get("EDGECONV_ABLATE", "0"))
    blocks, groups, TC = make_layout(sched_lo, sched_hi, BPC, gather_blocks)
    nc = bacc.Bacc("TRN2", num_swdge_queues=4)

    # ---- inputs ----
    xt = nc.declare_dram_parameter("xt", [C, NBT * C], BF16, isOutput=False)
    xt_own = nc.declare_dram_parameter("xt_own", [C, BPC * C], BF16, isOutput=False)
    wv_t = nc.declare_dram_parameter("wv_t", [C, C], BF16, isOutput=False)
    wu_t = nc.declare_dram_parameter("wu_t", [C, C], BF16, isOutput=False)
    ba = nc.declare_dram_parameter("ba", [1, C], BF16, isOutput=False)
    wb2 = nc.declare_dram_parameter("wb2", [C, C], BF16, isOutput=False)
    c0 = nc.declare_dram_parameter("c0", [1, C], BF16, isOutput=False)
    iota_col = nc.declare_dram_parameter("iota_col", [C, 1], F32, isOutput=False)
    iota_row4 = nc.declare_dram_parameter("iota_row4", [C, GRP, C], BF16, isOutput=False)
    ident = nc.declare_dram_parameter("ident", [C, C], BF16, isOutput=False)
    deg = nc.declare_dram_parameter("deg", [1, BPC * C], BF16, isOutput=False)
    ones_row = nc.declare_dram_parameter("ones_row", [1, C], BF16, isOutput=False)
    src16 = nc.declare_dram_parameter("src16", [128, TC * 8], I16, isOutput=False)
    dst_row = nc.declare_dram_parameter("dst_row", [1, TC * C], BF16, isOutput=False)
    dst_col = nc.declare_dram_parameter("dst_col", [C, TC], BF16, isOutput=False)
    out_nm = nc.declare_dram_parameter("out_nm", [BPC * C, C], F32, isOutput=True)

    max_gchunks = max((nlo + nhi for _, nlo, nhi, _ in groups), default=1)

    with tile.TileContext(nc) as tc:
        with (
            tc.tile_pool(name="persist", bufs=1) as persist,
            tc.tile_pool(name="nodeio", bufs=2) as nodeio,
            tc.tile_pool(name="edgeio", bufs=2) as edgeio,
            tc.tile_pool(name="sbuild", bufs=3) as sbuild,
            tc.tile_pool(name="tbuf", bufs=3) as tbuf,
            tc.tile_pool(name="outio", bufs=2) as outio,
            tc.tile_pool(name="widep", bufs=3, space="PSUM") as widep,
            tc.tile_pool(name="tpp", bufs=2, space="PSUM") as tpp,
            tc.tile_pool(name="aggp", bufs=1, space="PSUM") as aggp,
        ):
            # ---- persistent SBUF state ----
            nlo_elems = min(SPLIT, NBT * C)
            nhi_elems = max(NBT * C - nlo_elems, 0)
            v_lo = persist.tile([128, nlo_elems], BF16, tag="vlo")
            v_hi = persist.tile([128, max(nhi_elems, C)], BF16, tag="vhi",
                                name="v_hi")

            def v_dst(q0, q1):
                """SBUF destination APs covering node blocks [q0, q1)."""
                e0, e1 = q0 * C, q1 * C
                parts = []
                if e0 < nlo_elems:
                    parts.append(v_lo[:, e0: min(e1, nlo_elems)])
                if e1 > nlo_elems:
                    parts.append(v_hi[:, max(e0 - nlo_elems, 0): e1 - nlo_elems])
                return parts

            u_sb = persist.tile([128, BPC * C], BF16, tag="u")
            srci_sb = persist.tile([128, TC * 8], I16, tag="srci")
            nc.sync.dma_start(out=srci_sb[:], in_=src16[:])
            dstc_sb = persist.tile([C, TC], BF16, tag="dstc")
            nc.sync.dma_start(out=dstc_sb[:], in_=dst_col[:])
            wv_sb = persist.tile([C, C], BF16, tag="wv")
            nc.sync.dma_start(out=wv_sb[:], in_=wv_t[:])
            wu_sb = persist.tile([C, C], BF16, tag="wu")
            nc.sync.dma_start(out=wu_sb[:], in_=wu_t[:])
            ba_sb = persist.tile([1, C], BF16, tag="ba")
            nc.sync.dma_start(out=ba_sb[:], in_=ba[:])
            wb2_sb = persist.tile([C, C], BF16, tag="wb2")
            nc.sync.dma_start(out=wb2_sb[:], in_=wb2[:])
            c0_sb = persist.tile([1, C], BF16, tag="c0")
            nc.sync.dma_start(out=c0_sb[:], in_=c0[:])
            ic_sb = persist.tile([C, 1], F32, tag="ic")
            nc.sync.dma_start(out=ic_sb[:], in_=iota_col[:])
            ir_sb = persist.tile([C, GRP, C], BF16, tag="ir")
            nc.sync.dma_start(out=ir_sb[:], in_=iota_row4[:])
            id_sb = persist.tile([C, C], BF16, tag="id")
            nc.sync.dma_start(out=id_sb[:], in_=ident[:])
            deg_sb = persist.tile([1, BPC * C], BF16, tag="deg")
            nc.sync.dma_start(out=deg_sb[:], in_=deg[:])
            onesr_sb = persist.tile([1, C], BF16, tag="onesr")
            nc.sync.dma_start(out=onesr_sb[:], in_=ones_row[:])

            # ================= node phase =================
            # v[n] for ALL nodes -> v_sb (node-major: partition n%128,
            # free bytes (n//128)*256). u[n] for own nodes -> u_sb.
            for g0 in range(0, NBT, node_grp):
                g1 = min(g0 + node_grp, NBT)
                xt_sb = nodeio.tile([C, node_grp * C], BF16, tag="xt")
                nc.sync.dma_start(out=xt_sb[:, : (g1 - g0) * C],
                                  in_=xt[:, g0 * C: g1 * C])
                for q0 in range(g0, g1, GRP):
                    q1 = min(q0 + GRP, g1)
                    vps = widep.tile([C, GRP * C], F32, tag="wide")
                    for b in range(q0, q1):
                        lhs = xt_sb[:, (b - g0) * C: (b - g0 + 1) * C]
                        nc.tensor.matmul(vps[:, (b - q0) * C: (b - q0 + 1) * C],
                                         lhsT=lhs, rhs=wv_sb[:],
                                         start=True, stop=True)
                    off = 0
                    for part in v_dst(q0, q1):
                        w = part.shape[-1]
                        if (q0 // GRP) % 2 == 0:
                            nc.vector.tensor_copy(
                                out=part, in_=vps[:, off: off + w])
                        else:
                            nc.scalar.activation(
                                out=part, in_=vps[:, off: off + w],
                                func=AF.Copy)
                        off += w

            for g0 in range(0, BPC, node_grp):
                g1 = min(g0 + node_grp, BPC)
                xo_sb = nodeio.tile([C, node_grp * C], BF16, tag="xo")
                nc.sync.dma_start(out=xo_sb[:, : (g1 - g0) * C],
                                  in_=xt_own[:, g0 * C: g1 * C])
                for q0 in range(g0, g1, GRP):
                    q1 = min(q0 + GRP, g1)
                    ups = widep.tile([C, GRP * C], F32, tag="wide")
                    for b in range(q0, q1):
                        lhs = xo_sb[:, (b - g0) * C: (b - g0 + 1) * C]
                        sl = slice((b - q0) * C, (b - q0 + 1) * C)
                        nc.tensor.matmul(ups[:, sl], lhsT=lhs, rhs=wu_sb[:],
                                         start=True, stop=False)
                        nc.tensor.matmul(ups[:, sl], lhsT=onesr_sb[:],
                                         rhs=ba_sb[:], start=False, stop=True)
                    nc.vector.tensor_copy(out=u_sb[:, q0 * C: q1 * C],
                                          in_=ups[:, : (q1 - q0) * C])

            # ================= edge phase =================
            grp_of_block = {}
            for gi_, (g_start, nlo_g, nhi_g, bs) in enumerate(groups):
                grp_of_block[bs[0]] = gi_

            vg_sb = None
            vg_base = 0
            gq = [0]

            # software pipeline: defer transpose+scatter of the previous
            # run while the next run's message matmul issues.
            pending = []

            def flush_pending():
                while pending:
                    pending.pop(0)()

            for b in range(BPC):
                lo0, nl, hi0, nh = blocks[b]
                nch = nl + nh
                if b in grp_of_block:
                    g_start, nlo_g, nhi_g, _ = groups[grp_of_block[b]]
                    vg_base = g_start
                    ng = nlo_g + nhi_g
                    if ng > 0:
                        vg_sb = edgeio.tile([128, max_gchunks * C], BF16,
                                            tag="vg")
                        if ablate == 1:
                            nc.gpsimd.memset(vg_sb[:], 0.0)
                        else:
                         for (cb, cn, lo_half) in ((0, nlo_g, True),
                                                  (nlo_g, nhi_g, False)):
                            in_ap = v_lo[:] if lo_half else v_hi[:]
                            for c0_ in range(0, cn, MAXCH):
                                cw = min(MAXCH, cn - c0_)
                                sl = g_start + cb + c0_
                                o0 = (cb + c0_) * C
                                nc.gpsimd.dma_gather(
                                    out_ap=vg_sb[:, o0: o0 + cw * C]
                                        .unsqueeze(1),
                                    in_ap=in_ap,
                                    idxs_ap=srci_sb[:, sl * 8: (sl + cw) * 8],
                                    num_idxs=cw * C,
                                    num_idxs_reg=cw * C,
                                    elem_size=C,
                                    transpose=True,
                                    sbuf_tokens_per_rank=128,
                                    sbuf_free_dim_per_rank=256,
                                    single_packet=SPKT,
                                    queue_num=gq[0] % 4)
                                gq[0] += 1
                        dstrg_sb = edgeio.tile([1, max_gchunks * C], BF16,
                                               tag="dstr")
                        nc.sync.dma_start(
                            out=dstrg_sb[:, : ng * C],
                            in_=dst_row[0:1, g_start * C: (g_start + ng) * C])

                if nch == 0:
                    flush_pending()
                    agg_sb = outio.tile([C, C], BF16, tag="aggsb")
                    nc.gpsimd.memset(agg_sb[:], 0.0)
                else:
                    aggT = aggp.tile([C, C], F32, tag="agg")
                    ch_done = [0]
                    for (r0, rn) in ((lo0, nl), (hi0, nh)):
                        for j0 in range(0, rn, GRP):
                            gw = min(GRP, rn - j0)
                            rel = r0 + j0 - vg_base
                            slot0 = r0 + j0
                            # one-hot builds for this run
                            bc_ps = widep.tile([C, GRP * C], F32, tag="wide")
                            nc.tensor.matmul(
                                bc_ps[:, : gw * C], lhsT=onesr_sb[:],
                                rhs=dstrg_sb[0:1, rel * C: (rel + gw) * C],
                                start=True, stop=True)
                            sT_sb = sbuild.tile([C, GRP * C], BF16, tag="sT")
                            nc.vector.tensor_scalar(
                                out=sT_sb[:, : gw * C],
                                in0=bc_ps[:, : gw * C],
                                scalar1=ic_sb[:], scalar2=None,
                                op0=mybir.AluOpType.is_equal)
                            s_sb = sbuild.tile([C, GRP, C], BF16, tag="s")
                            nc.vector.tensor_tensor(
                                out=s_sb[:, :gw, :],
                                in0=ir_sb[:, :gw, :],
                                in1=dstc_sb[:, slot0: slot0 + gw]
                                    .to_broadcast([C, gw, C]),
                                op=mybir.AluOpType.is_equal)
                            # message (feature-major): u[dst] + v[src]
                            msg_ps = widep.tile([C, GRP * C], F32, tag="wide")
                            nc.tensor.matmul(
                                msg_ps[:, : gw * C],
                                lhsT=u_sb[:, b * C: (b + 1) * C],
                                rhs=sT_sb[:, : gw * C],
                                start=True, stop=False)
                            nc.tensor.matmul(
                                msg_ps[:, : gw * C], lhsT=id_sb[:],
                                rhs=vg_sb[:, rel * C: (rel + gw) * C],
                                start=False, stop=True)
                            tT_sb = tbuf.tile([C, GRP * C], BF16, tag="tT")
                            nc.scalar.activation(out=tT_sb[:, : gw * C],
                                                 in_=msg_ps[:, : gw * C],
                                                 func=AF.Relu)

                            def finish(tT_sb=tT_sb, s_sb=s_sb, gw=gw,
                                       aggT=aggT, ch_done=ch_done, nch=nch):
                                t_sb = tbuf.tile([C, GRP * C], BF16, tag="t")
                                if ablate == 2:
                                    nc.vector.tensor_copy(
                                        out=t_sb[:, : gw * C],
                                        in_=tT_sb[:, : gw * C])
                                else:
                                    tp_ps = tpp.tile([C, GRP * C], BF16, tag="tp")
                                    for j in range(gw):
                                        nc.tensor.matmul(
                                            tp_ps[:, j * C: (j + 1) * C],
                                            lhsT=tT_sb[:, j * C: (j + 1) * C],
                                            rhs=id_sb[:], is_transpose=True,
                                            start=True, stop=True)
                                    nc.vector.tensor_copy(out=t_sb[:, : gw * C],
                                                          in_=tp_ps[:, : gw * C])
                                for j in range(gw):
                                    nc.tensor.matmul(
                                        aggT[:],
                                        lhsT=t_sb[:, j * C: (j + 1) * C],
                                        rhs=s_sb[:, j, :],
                                        start=(ch_done[0] == 0),
                                        stop=(ch_done[0] == nch - 1))
                                    ch_done[0] += 1

                            flush_pending()
                            pending.append(finish)
                    flush_pending()
                    agg_sb = outio.tile([C, C], BF16, tag="aggsb")
                    nc.vector.tensor_copy(out=agg_sb[:], in_=aggT[:])

                # conv: node-major [n, c]
                cps = aggp.tile([C, C], F32, tag="conv")
                nc.tensor.matmul(cps[:], lhsT=agg_sb[:], rhs=wb2_sb[:],
                                 start=True, stop=False)
                nc.tensor.matmul(cps[:], lhsT=deg_sb[0:1, b * C: (b + 1) * C],
                                 rhs=c0_sb[:], start=False, stop=True)

                o_sb = outio.tile([C, C], F32, tag="o")
                if apply_norm:
                    h_sb = outio.tile([C, C], F32, tag="h")
                    nc.scalar.activation(out=h_sb[:], in_=cps[:],
                                         func=AF.Relu)
                    sq_sb = outio.tile([C, C], BF16, tag="sq")
                    nrm = outio.tile([C, 4], F32, tag="nrm")
                    nc.scalar.activation(out=sq_sb[:], in_=h_sb[:],
                                         func=AF.Square,
                                         accum_out=nrm[:, 0:1])
                    nc.scalar.activation(out=nrm[:, 1:2], in_=nrm[:, 0:1],
                                         func=AF.Sqrt)
                    nc.vector.tensor_scalar(out=nrm[:, 2:3], in0=nrm[:, 1:2],
                                            scalar1=1e-12, scalar2=None,
                                            op0=mybir.AluOpType.max)
                    nc.vector.reciprocal(out=nrm[:, 3:4], in_=nrm[:, 2:3])
                    nc.scalar.activation(out=o_sb[:], in_=h_sb[:],
                                         func=AF.Copy, scale=nrm[:, 3:4])
                else:
                    nc.scalar.activation(out=o_sb[:], in_=cps[:],
                                         func=AF.Copy)
                nc.sync.dma_start(out=out_nm[b * C: (b + 1) * C, :],
                                  in_=o_sb[:])

    nc.compile()
    return nc


# ---------------- host-side data prep ----------------


def make_layout(sched_lo, sched_hi, bpc, gather_blocks=2):
    """Group-major slot order: per gather group, all lo slots (block-major)
    then all hi slots. Returns per-block (lo_start, nlo, hi_start, nhi),
    group list (chunk_start, nlo_g, nhi_g, blocks)."""
    blocks = []
    groups = []
    pos = 0
    b = 0
    while b < bpc:
        bs = list(range(b, min(b + gather_blocks, bpc)))
        g_start = pos
        lo_starts = {}
        for bb in bs:
            lo_starts[bb] = pos
            pos += sched_lo[bb]
        nlo_g = pos - g_start
        hi_starts = {}
        for bb in bs:
            hi_starts[bb] = pos
            pos += sched_hi[bb]
        nhi_g = pos - g_start - nlo_g
        for bb in bs:
            blocks.append((lo_starts[bb], sched_lo[bb],
                           hi_starts[bb], sched_hi[bb]))
        groups.append((g_start, nlo_g, nhi_g, bs))
        b += gather_blocks
    return blocks, groups, pos


def prep_edges(src, dst, n_cores, bpc, gather_blocks=2):
    """Partition edges by dst core/block, split each block's edges into
    lo (src < SPLIT) and hi chunks for int16 dma_gather indexing."""
    npc = bpc * C
    order = np.argsort(dst, kind="stable")
    src_s, dst_s = src[order], dst[order]
    core_lists = []
    nlo = np.zeros((n_cores, bpc), np.int64)
    nhi = np.zeros((n_cores, bpc), np.int64)
    for k in range(n_cores):
        lo_ = np.searchsorted(dst_s, k * npc, side="left")
        hi_ = np.searchsorted(dst_s, (k + 1) * npc, side="left")
        s_k, d_k = src_s[lo_:hi_], dst_s[lo_:hi_] - k * npc
        blk = d_k // C
        per_blk = []
        for b in range(bpc):
            m = blk == b
            sb, db = s_k[m], d_k[m] - b * C
            isl = sb < SPLIT
            per_blk.append(((sb[isl], db[isl]), (sb[~isl], db[~isl])))
            nlo[k, b] = isl.sum()
            nhi[k, b] = (~isl).sum()
        core_lists.append(per_blk)
    sched_lo = [int(x) for x in np.ceil(nlo.max(axis=0) / C).astype(np.int64)]
    sched_hi = [int(x) for x in np.ceil(nhi.max(axis=0) / C).astype(np.int64)]
    blocks, groups, TC = make_layout(sched_lo, sched_hi, bpc, gather_blocks)

    per_core = []
    for k in range(n_cores):
        si16 = np.zeros((16, TC * 8), np.int16)
        db_ = np.full((TC, C), 200.0, np.float64)
        for b in range(bpc):
            (slo, sdlo), (shi, sdhi) = core_lists[k][b]
            lo0, nl, hi0, nh = blocks[b]
            for (vals, dvals, base, nslots, off) in (
                    (slo, sdlo, lo0, nl, 0), (shi, sdhi, hi0, nh, SPLIT)):
                n = len(vals)
                if nslots == 0:
                    continue
                idx = np.arange(n)
                ch = base + idx // C
                lane = idx % C
                iv = (vals - off).astype(np.int16)
                si16[lane % 16, ch * 8 + lane // 16] = iv
                db_[ch, lane] = dvals
        full = np.zeros((128, TC * 8), np.int16)
        for rr in range(8):
            full[rr * 16: (rr + 1) * 16] = si16
        per_core.append({
            "src16": full,
            "dst_col": np.ascontiguousarray(db_.T.astype(BF16_NP)),
            "dst_row": np.ascontiguousarray(
                db_.reshape(1, -1).astype(BF16_NP)),
        })
    return sched_lo, sched_hi, per_core


def fold_weights(wa, ba_, g, be, rm, rv, wb, bb, bn_eps=1e-5):
    wa = wa.astype(np.float64)
    A_i, A_j = wa[:, :C], wa[:, C:]
    s = g.astype(np.float64) / np.sqrt(rv.astype(np.float64) + bn_eps)
    wb64 = wb.astype(np.float64)
    wu_t = (A_i - A_j).T
    wv_t = A_j.T
    wb2 = s[:, None] * wb64.T          # wb2[j, i] = s_j * wb[i, j]
    c0 = bb.astype(np.float64) + (be.astype(np.float64) - rm.astype(np.float64) * s) @ wb64.T
    return (wu_t.astype(BF16_NP), wv_t.astype(BF16_NP),
            ba_.astype(BF16_NP).reshape(1, C),
            wb2.astype(BF16_NP), c0.astype(BF16_NP).reshape(1, C))


def make_consts():
    ic = np.arange(C, dtype=np.float32).reshape(C, 1)
    ir4 = np.tile(np.arange(C, dtype=np.float64), (C, GRP, 1)).astype(BF16_NP)
    ident = np.eye(C, dtype=np.float64).astype(BF16_NP)
    return ic, ir4, ident


# ======================================================================
# Full-problem kernel: 2-layer EdgeConv encoder, N=50000, E=600000, C=128
# ======================================================================

import os

N_NODES = 50000
N_EDGES = 600000
CORES = 8
BPC = 49                  # blocks per core
NBT = CORES * BPC         # 392 blocks total
NP = NBT * C              # padded node count 50176
BN_EPS = 1e-5

LAST = {}                 # timing/info stash for test harness


def _prep_all(x, edge_index):
    src = np.asarray(edge_index[0], np.int64).astype(np.int32)
    dst = np.asarray(edge_index[1], np.int64).astype(np.int32)
    sched_lo, sched_hi, per_core = prep_edges(src, dst, CORES, BPC)
    deg_full = np.bincount(dst, minlength=NP).astype(np.float64)
    x_pad = np.zeros((NP, C), np.float32)
    x_pad[:N_NODES] = x
    xt = np.ascontiguousarray(x_pad.T).astype(BF16_NP)
    return sched_lo, sched_hi, per_core, deg_full, xt


def _layer_inputs(xt_bf16, per_core, deg_full, wset):
    wu_t, wv_t, ba_f, wb2, c0 = wset
    ic, ir4, ident = make_consts()
    onesr = np.ones((1, C), dtype=BF16_NP)
    in_maps = []
    for k in range(CORES):
        npc = BPC * C
        in_maps.append({
            "xt": xt_bf16,
            "xt_own": np.ascontiguousarray(xt_bf16[:, k * npc: (k + 1) * npc]),
            "wv_t": wv_t, "wu_t": wu_t, "ba": ba_f, "wb2": wb2, "c0": c0,
            "iota_col": ic, "iota_row4": ir4, "ident": ident,
            "deg": np.ascontiguousarray(
                deg_full[k * npc: (k + 1) * npc].reshape(1, npc).astype(BF16_NP)),
            "ones_row": onesr,
            "src16": per_core[k]["src16"],
            "dst_row": per_core[k]["dst_row"],
            "dst_col": per_core[k]["dst_col"],
        })
    return in_maps


_NTFF_HOOK = None


def _get_ntff_hook():
    global _NTFF_HOOK
    if _NTFF_HOOK is None:
        sys.path.insert(0, "/root/.axon_site")
        from trn_agent_boot.trn_boot import _ntff_profile_via_ctypes
        _NTFF_HOOK = _ntff_profile_via_ctypes("/opt/axon/libaxon_pjrt.so")
    return _NTFF_HOOK


def _run(nc, in_maps):
    import tempfile
    from concourse import bass2jax
    trace = bool(int(os.environ.get("EDGECONV_TRACE", "0")))
    hook = _get_ntff_hook() if trace else None
    if hook is None:
        results = bass2jax.run_bass_via_pjrt(nc, in_maps, n_cores=CORES)
        LAST.setdefault("exec_ns", []).append(None)
        return results
    neff_dir = tempfile.mkdtemp(prefix="edgeconv_ntff_")
    with hook(neff_dir, [0]):
        results = bass2jax.run_bass_via_pjrt(nc, in_maps, n_cores=CORES)
    exec_ns = None
    try:
        import glob as _glob
        import gauge.profiler
        from concourse._compat import FishPath
        if _glob.glob(os.path.join(neff_dir, "*_body*.ntff")):
            profile = gauge.profiler.Profile(
                profile_path=FishPath(neff_dir), kernel_dev_mode=True,
                profile_on_exit=False, bass_kernel=nc.m,
                offline_processing=True, fname="*_body*")
            pr = profile.to_perfetto(model_index=(0,))
            if pr:
                exec_ns = pr[0].exec_time_ns
                LAST.setdefault("trace_paths", []).append(pr[0].trace_path)
    except Exception as e:  # profiling must never break the kernel
        LAST.setdefault("trace_errors", []).append(repr(e))
    LAST.setdefault("neff_dirs", []).append(neff_dir)
    LAST.setdefault("exec_ns", []).append(exec_ns)
    return results


def kernel(**inputs):
    x = np.asarray(inputs["x"], np.float32)
    edge_index = np.asarray(inputs["edge_index"])
    sched_lo, sched_hi, per_core, deg_full, xt = _prep_all(x, edge_index)

    w1 = fold_weights(np.asarray(inputs["w1a"]), np.asarray(inputs["b1a"]),
                      np.asarray(inputs["g1"]), np.asarray(inputs["be1"]),
                      np.asarray(inputs["rm1"]), np.asarray(inputs["rv1"]),
                      np.asarray(inputs["w1b"]), np.asarray(inputs["b1b"]),
                      BN_EPS)
    w2 = fold_weights(np.asarray(inputs["w2a"]), np.asarray(inputs["b2a"]),
                      np.asarray(inputs["g2"]), np.asarray(inputs["be2"]),
                      np.asarray(inputs["rm2"]), np.asarray(inputs["rv2"]),
                      np.asarray(inputs["w2b"]), np.asarray(inputs["b2b"]),
                      BN_EPS)

    nc1 = build_layer(NBT, BPC, sched_lo, sched_hi, apply_norm=True)
    r1 = _run(nc1, _layer_inputs(xt, per_core, deg_full, w1))
    h = np.concatenate([np.asarray(r["out_nm"], np.float32) for r in r1],
                       axis=0)                        # [NP, C] node-major
    xt2 = np.ascontiguousarray(h.T).astype(BF16_NP)   # [C, NP] feature-major

    nc2 = build_layer(NBT, BPC, sched_lo, sched_hi, apply_norm=False)
    r2 = _run(nc2, _layer_inputs(xt2, per_core, deg_full, w2))
    out = np.concatenate([np.asarray(r["out_nm"], np.float32) for r in r2],
                         axis=0)
    return np.ascontiguousarray(out[:N_NODES]).astype(np.float32)
